# revision 4
# baseline (speedup 1.0000x reference)
"""Multi-head self-attention Trainium2 kernel (8 NeuronCores, batch-parallel).

Reference: qkv = x @ W_qkv + b; 12-head scaled-dot-product attention; concat.
Shapes: x[8,1024,768], W_qkv[768,2304], b_qkv[2304] -> out[8,1024,768].
Sharding: one batch element per core; W/b replicated to all cores.

Per-core dataflow:
  x --PE transpose--> xT[768,1024] (f32r), copies batched 4 chunks at a time
  qk tiles (bf16): per (f-block, token-half) [128,512] = W-block(lhsT) @ xT
    produced in N>=256 slices; Q/K biases added on the PSUM->SBUF copy
  V[128,12,66] bf16 per token chunk (strips of 4 heads; col 64 = ones)
  per (pair p, q-half qh), per key-chunk kc:
    scT[128,2,512] = K-slice(lhsT) @ Q-half  (2 row-tiled MMs, one per head)
    ex[128,2,512] bf16 = ACT Exp(0.125 * scT)   (scale folded into ACT)
    av[q=128,65] += ex-chunk(lhsT) @ [V_h|1]  bf16 N=65 MMs, accumulated
      over kc; av already in [q, feature] orientation, col 64 = denominator
  normalize: rc = 1/av[:,:,64] (DVE), onat[:, c, h*64:...] = av * rc
  out DMA per chunk once the last pair finishes its q-half.

Scheduling: W is DMA'd in priority order (pair-0 Q/K columns, V strip 0,
then later pairs); QK-tile and V-strip production is spread across the
pair loop just-in-time so PE work per key-chunk stays balanced against
the ACT exp stream (ACT is the co-bottleneck at ~1038ns per key-chunk).
"""

import contextlib
import json as _json

import numpy as np

import concourse.bass as bass
import concourse.mybir as mybir
import concourse.tile as tile
from concourse.bass_utils import run_bass_kernel_spmd
from concourse.masks import make_identity

# --- BIR sync-wait legalization ------------------------------------------
# walrus's codegen in this toolchain accepts only one sync-wait command per
# instruction. Split every multi-wait instruction into N-1 preceding
# single-wait EventSemaphore instructions on the same engine.


def _legalize_sync_waits(bir_json: bytes) -> bytes:
    m = _json.loads(bir_json)
    ctr = 0
    for fn in m["functions"]:
        for bb in fn["blocks"]:
            out = []
            for ins in bb["instructions"]:
                si = ins.get("sync_info")
                waits = si.get("on_wait", []) if si else []
                if len(waits) > 1:
                    for w in waits[:-1]:
                        ctr += 1
                        out.append(
                            {
                                "debug": ins.get("debug", 0),
                                "engine": ins["engine"],
                                "ins": [],
                                "outs": [],
                                "name": f"evw-split-{ctr}",
                                "opcode": "EventSemaphore",
                                "sync_info": {"on_update": [], "on_wait": [w]},
                            }
                        )
                    si["on_wait"] = [waits[-1]]
                out.append(ins)
            bb["instructions"] = out
    return _json.dumps(m).encode()


_fixup_installed = False


def _install_bir_fixup():
    global _fixup_installed
    if _fixup_installed:
        return
    _fixup_installed = True
    import concourse.bass_utils as _bu

    _orig = _bu.compile_bir_kernel

    def _patched(bir_json, tmpdir, neff_name="file.neff"):
        if isinstance(bir_json, str):
            bir_json = bir_json.encode()
        return _orig(_legalize_sync_waits(bir_json), tmpdir, neff_name)

    _bu.compile_bir_kernel = _patched
    try:
        import concourse.bass2jax as _b2j

        _b2j.compile_bir_kernel = _patched
    except ImportError:
        pass


_install_bir_fixup()

B, N, D, H = 8, 1024, 768, 12
HD = D // H            # 64
F3 = 3 * D             # 2304
NCORE = 8
P = 128
NCHUNK = N // P        # 8 token chunks
KD = D // P            # 6 d_in chunks
QH = 512               # q-half size
NPAIR = H // 2         # 6
VW = HD + 1            # 65 (V cols + denominator ones col)
VPAD = 66              # padded per-head V width (4-byte aligned bf16)

f32 = mybir.dt.float32
f32r = mybir.dt.float32r
bf16 = mybir.dt.bfloat16
FT = mybir.ActivationFunctionType
ALU = mybir.AluOpType


def build_attention_nc():
    nc = bass.Bass()
    x_d = nc.declare_dram_parameter("x", [N, D], f32, isOutput=False)
    w_d = nc.declare_dram_parameter("W_qkv", [D, F3], f32, isOutput=False)
    b_d = nc.declare_dram_parameter("b_qkv", [F3], f32, isOutput=False)
    o_d = nc.declare_dram_parameter("out", [N, D], f32, isOutput=True)

    with tile.TileContext(nc) as tc, contextlib.ExitStack() as ctx:
        singles = ctx.enter_context(tc.tile_pool(name="singles", bufs=1))
        xpool = ctx.enter_context(tc.tile_pool(name="xpool", bufs=NCHUNK))
        xtpool = ctx.enter_context(tc.tile_pool(name="xtpool", bufs=KD))
        wpool = ctx.enter_context(tc.tile_pool(name="wpool", bufs=KD))
        qkpool = ctx.enter_context(tc.tile_pool(name="qkpool", bufs=10))
        vpool = ctx.enter_context(tc.tile_pool(name="vpool", bufs=NCHUNK))
        exppool = ctx.enter_context(tc.tile_pool(name="exppool", bufs=3))
        recpool = ctx.enter_context(tc.tile_pool(name="recpool", bufs=4))

        # PSUM budget (8 banks): wk [128,512] x2 = 2; sc [128,2,512] x2 = 4;
        # av [128,4,66] x2 = 2.
        wkps = ctx.enter_context(tc.tile_pool(name="wkps", bufs=2, space="PSUM"))
        scps = ctx.enter_context(tc.tile_pool(name="scps", bufs=2, space="PSUM"))
        avps = ctx.enter_context(tc.tile_pool(name="avps", bufs=2, space="PSUM"))

        def wk_psum():
            return wkps.tile([P, QH], f32, tag="wk", name="wktile")

        # ------------- constants -------------------------------------------
        ident = singles.tile([P, P], f32)
        make_identity(nc, ident)  # gpsimd

        ident_r = singles.tile([P, P], f32r)
        nc.vector.tensor_copy(out=ident_r, in_=ident)

        ones_f32 = singles.tile([P, 1], f32)
        nc.vector.memset(ones_f32, 1.0)
        ones_row_st = singles.tile([1, P], f32)
        nc.vector.memset(ones_row_st, 1.0)
        ones_row = singles.tile([1, P], f32r)
        nc.vector.tensor_copy(out=ones_row, in_=ones_row_st)

        # dummy exp to trigger the ACT table load early
        actwarm = singles.tile([1, 2], f32)
        nc.vector.memset(actwarm, 0.0)
        nc.scalar.activation(actwarm, actwarm, FT.Exp)

        b_sb = singles.tile([P, F3 // P], f32)
        nc.sync.dma_start(out=b_sb, in_=b_d[:].rearrange("(t p) -> p t", p=P))

        bv_st = singles.tile([1, D], f32)
        nc.sync.dma_start(out=bv_st, in_=b_d[2 * D : 3 * D][None, :])
        bv_sb = singles.tile([1, D], f32r)
        nc.vector.tensor_copy(out=bv_sb, in_=bv_st)

        # ------------- input DMAs (priority order) -------------------------
        x_sb = []
        for c in range(NCHUNK):
            t = xpool.tile([P, D], f32r, tag="x", name=f"x{c}")
            x_sb.append(t)

        def dma_x(c):
            nc.sync.dma_start(
                out=x_sb[c], in_=x_d[c * P : (c + 1) * P, :].bitcast(f32r)
            )

        w_sb = [wpool.tile([P, F3], f32r, tag="w", name=f"w{k}") for k in range(KD)]

        def dma_w_cols(f0, fw):
            for k in range(KD):
                nc.sync.dma_start(
                    out=w_sb[k][:, f0 : f0 + fw],
                    in_=w_d[k * P : (k + 1) * P, f0 : f0 + fw].bitcast(f32r),
                )

        for c in range(4):
            dma_x(c)
        dma_w_cols(0 * P, P)          # pair-0 Q cols
        dma_w_cols(6 * P, P)          # pair-0 K cols
        dma_w_cols(2 * D, 2 * P)      # V strip 0 (heads 0-3)
        for c in range(4, NCHUNK):
            dma_x(c)
        dma_w_cols(1 * P, P)          # pair-1 Q
        dma_w_cols(7 * P, P)          # pair-1 K
        dma_w_cols(2 * D + 2 * P, 2 * P)   # V strip 1 (heads 4-7)
        dma_w_cols(2 * P, P)
        dma_w_cols(8 * P, P)
        dma_w_cols(2 * D + 4 * P, 2 * P)   # V strip 2 (heads 8-11)
        for p in range(3, NPAIR):
            dma_w_cols(p * P, P)
            dma_w_cols((6 + p) * P, P)

        # ------------- x^T (PE transposes, batched copies) ------------------
        xt = [xtpool.tile([P, N], f32r, tag="xt", name=f"xt{k}") for k in range(KD)]

        def transpose_chunks(c0):
            # transpose x chunks c0..c0+3 into xt[k][:, c0*P:(c0+4)*P]
            for k in range(KD):
                ps = wk_psum()
                for j in range(4):
                    nc.tensor.transpose(
                        ps[:, j * P : (j + 1) * P].bitcast(f32r),
                        x_sb[c0 + j][:, k * P : (k + 1) * P],
                        ident_r,
                    )
                nc.vector.tensor_copy(
                    out=xt[k][:, c0 * P : (c0 + 4) * P], in_=ps.bitcast(f32r)
                )

        # broadcast b_v across partitions once: bvb[p, f] = b_v[f]
        bvb = singles.tile([P, D], f32)
        for f0 in range(0, D, 256):
            ps = wk_psum()[:, 0:256]
            nc.tensor.matmul(
                ps, ones_row, bv_sb[:, f0 : f0 + 256], start=True, stop=True
            )
            nc.vector.tensor_copy(out=bvb[:, f0 : f0 + 256], in_=ps)

        # ------------- qk tiles ---------------------------------------------
        # qk[(f, half)]: [128, 512] bf16; partitions = features f*128..+128,
        # cols = tokens half*512..+512. f 0..5 = Q blocks, 6..11 = K blocks.
        qk_tiles = {}

        def get_qk(f, half):
            key = (f, half)
            if key not in qk_tiles:
                qk_tiles[key] = qkpool.tile(
                    [P, QH], bf16, tag="qk", name=f"qk{f}_{half}"
                )
            return qk_tiles[key]

        def make_qk(f, half, n0=0, nw=QH):
            # produce token-cols [n0, n0+nw) of tile (f, half); nw >= 256
            t = get_qk(f, half)
            ps = wk_psum()[:, 0:nw]
            for k in range(KD):
                nc.tensor.matmul(
                    ps,
                    w_sb[k][:, f * P : (f + 1) * P],
                    xt[k][:, half * QH + n0 : half * QH + n0 + nw],
                    start=(k == 0),
                    stop=(k == KD - 1),
                )
            nc.vector.tensor_scalar_add(
                t[:, n0 : n0 + nw], ps, b_sb[:, f : f + 1]
            )
            return t

        # ------------- V tiles ----------------------------------------------
        # v[c]: [128, 12, 66] bf16; [:, h, 0:64] = V for head h, [:, h, 64] = 1
        v_sb = []
        for c in range(NCHUNK):
            t = vpool.tile([P, H, VPAD], bf16, tag="v", name=f"v{c}")
            v_sb.append(t)

        def make_v(c, s):
            # strip s covers heads 4s..4s+4 (f-cols 2D + s*256 ..+256)
            if s == 0:
                nc.vector.tensor_copy(
                    out=v_sb[c][:, :, HD : HD + 1],
                    in_=ones_f32[:, 0:1, None].to_broadcast([P, H, 1]),
                )
            f0 = s * 256
            ps = wk_psum()[:, 0:256]
            for k in range(KD):
                nc.tensor.matmul(
                    ps,
                    xt[k][:, c * P : (c + 1) * P],
                    w_sb[k][:, 2 * D + f0 : 2 * D + f0 + 256],
                    start=(k == 0),
                    stop=(k == KD - 1),
                )
            nc.vector.tensor_tensor(
                v_sb[c][:, 4 * s : 4 * s + 4, 0:HD],
                ps.rearrange("p (h d) -> p h d", d=HD),
                bvb[:, f0 : f0 + 256].rearrange("p (h d) -> p h d", d=HD),
                ALU.add,
            )

        # ------------- bootstrap: transposes + first tiles ------------------
        transpose_chunks(0)
        make_qk(0, 0)                   # qt(pair0, qh0)
        make_qk(6, 0, 0, 256)           # kt(pair0) tokens 0:256
        make_qk(6, 0, 256, 256)         # kt(pair0) tokens 256:512

        onat = singles.tile([P, NCHUNK, D], f32)

        # JIT work queue: list of thunks, two popped per key-chunk iteration
        jit_q = []

        def run_jit(n):
            for _ in range(n):
                if jit_q:
                    jit_q.pop(0)()

        # pair 0 qh0 extra work: V strip0 chunks are emitted inline (AV needs
        # them); remaining bootstrap goes on the jit queue.
        jit_q.append(lambda: transpose_chunks(4))
        jit_q.append(lambda: make_qk(6, 1, 0, 256))
        jit_q.append(lambda: make_qk(6, 1, 256, 256))
        jit_q.append(lambda: make_qk(0, 1))

        # schedule of deferred production work, per (pair, qh):
        #   pair p qh0 -> kt(pair p+1) halves; pair p qh1 -> qt(p+1) halves
        #   V strips: strip1 over p0qh1+p1qh0, strip2 over p1qh1..p2qh1
        def sched(p, qh):
            w = []
            if p + 1 < NPAIR:
                if qh == 0:
                    w.append(lambda: make_qk(6 + p + 1, 0))
                    w.append(lambda: make_qk(6 + p + 1, 1))
                else:
                    w.append(lambda: make_qk(p + 1, 0))
                    w.append(lambda: make_qk(p + 1, 1))
            if (p, qh) == (0, 1):
                for c in range(4):
                    w.append(lambda c=c: make_v(c, 1))
            elif (p, qh) == (1, 0):
                for c in range(4, NCHUNK):
                    w.append(lambda c=c: make_v(c, 1))
            elif (p, qh) == (1, 1):
                for c in range(4):
                    w.append(lambda c=c: make_v(c, 2))
            elif (p, qh) == (2, 0):
                for c in range(4, NCHUNK):
                    w.append(lambda c=c: make_v(c, 2))
            return w

        # ------------- attention pair loop ----------------------------------
        for p in range(NPAIR):
            qt = [get_qk(p, 0), get_qk(p, 1)]
            kt = [get_qk(6 + p, 0), get_qk(6 + p, 1)]

            for qh in range(2):
                jit_q.extend(sched(p, qh))
                av = [
                    avps.tile([P, 4, VPAD], f32, tag="av", name=f"av{hi}")
                    for hi in range(2)
                ]
                for kc in range(NCHUNK):
                    kth = kt[kc // 4]
                    kcol = (kc % 4) * P
                    sc = scps.tile([P, 2, QH], f32, tag="sc", name="sc")
                    for hi in range(2):
                        nc.tensor.matmul(
                            sc[:, hi, :],
                            kth[64 * hi : 64 * hi + 64, kcol : kcol + P],
                            qt[qh][64 * hi : 64 * hi + 64, :],
                            start=True,
                            stop=True,
                            tile_position=(64 * hi, 0),
                        )
                    if p == 0 and qh == 0:
                        make_v(kc, 0)  # strip 0 JIT (AV below needs it)
                        run_jit(1)
                    else:
                        run_jit(2)
                    ex = exppool.tile([P, 2, QH], bf16, tag="exp", name="ex")
                    nc.scalar.activation(ex, sc, FT.Exp, scale=0.125)
                    for hi in range(2):
                        for qc in range(4):
                            # one bank-clearing start per av bank: later
                            # first-writes overwrite per-element (has_written
                            # cleared), later kc's accumulate
                            nc.tensor.matmul(
                                av[hi][:, qc, 0:VW],
                                ex[:, hi, qc * P : (qc + 1) * P],
                                v_sb[kc][:, 2 * p + hi, 0:VW],
                                start=(kc == 0 and qc == 0),
                                stop=(kc == NCHUNK - 1 and qc == 3),
                                skip_group_check=True,
                            )
                # normalize into onat: rc = 1/denominator, out = av * rc
                for hi in range(2):
                    h = 2 * p + hi
                    rc = recpool.tile([P, 4], f32, tag="rec", name="rc")
                    nc.vector.reciprocal(out=rc, in_=av[hi][:, :, HD])
                    nc.vector.tensor_tensor(
                        onat[:, qh * 4 : (qh + 1) * 4, h * HD : (h + 1) * HD],
                        av[hi][:, :, 0:HD],
                        rc[:, :, None].to_broadcast([P, 4, HD]),
                        ALU.mult,
                    )
                if p == NPAIR - 1:
                    for c in range(qh * 4, (qh + 1) * 4):
                        nc.sync.dma_start(
                            out=o_d[c * P : (c + 1) * P, :], in_=onat[:, c, :]
                        )

    return nc


def kernel(x: np.ndarray, W_qkv: np.ndarray, b_qkv: np.ndarray) -> np.ndarray:
    nc = build_attention_nc()
    in_maps = [
        {
            "x": np.ascontiguousarray(x[c], dtype=np.float32),
            "W_qkv": np.ascontiguousarray(W_qkv, dtype=np.float32),
            "b_qkv": np.ascontiguousarray(b_qkv, dtype=np.float32),
        }
        for c in range(NCORE)
    ]
    res = run_bass_kernel_spmd(nc, in_maps, core_ids=list(range(NCORE)))
    return np.stack([res.results[c]["out"] for c in range(NCORE)], axis=0)


# revision 5
# speedup vs baseline: 1.0318x; 1.0318x over previous
"""Multi-head self-attention Trainium2 kernel (8 NeuronCores, batch-parallel).

Reference: qkv = x @ W_qkv + b; 12-head scaled-dot-product attention; concat.
Shapes: x[8,1024,768], W_qkv[768,2304], b_qkv[2304] -> out[8,1024,768].
Sharding: one batch element per core; W/b replicated to all cores.

Per-core dataflow:
  x --PE transpose--> xT[768,1024] (f32r), copies batched 4 chunks at a time
  qk tiles (bf16): per (f-block, token-half) [128,512] = W-block(lhsT) @ xT
    produced in N>=256 slices; Q/K biases added on the PSUM->SBUF copy
  V[128,12,66] bf16 per token chunk (strips of 4 heads; col 64 = ones)
  per (pair p, q-half qh), per key-chunk kc:
    scT[128,2,512] = K-slice(lhsT) @ Q-half  (2 row-tiled MMs, one per head)
    ex[128,2,512] bf16 = ACT Exp(0.125 * scT)   (scale folded into ACT)
    av[q=128,65] += ex-chunk(lhsT) @ [V_h|1]  bf16 N=65 MMs, accumulated
      over kc; av already in [q, feature] orientation, col 64 = denominator
  normalize: rc = 1/av[:,:,64] (DVE), onat[:, c, h*64:...] = av * rc
  out DMA per chunk once the last pair finishes its q-half.

Scheduling: W is DMA'd in priority order (pair-0 Q/K columns, V strip 0,
then later pairs); QK-tile and V-strip production is spread across the
pair loop just-in-time so PE work per key-chunk stays balanced against
the ACT exp stream (ACT is the co-bottleneck at ~1038ns per key-chunk).
"""

import contextlib
import json as _json

import numpy as np

import concourse.bass as bass
import concourse.mybir as mybir
import concourse.tile as tile
from concourse.bass_utils import run_bass_kernel_spmd
from concourse.masks import make_identity

# --- BIR sync-wait legalization ------------------------------------------
# walrus's codegen in this toolchain accepts only one sync-wait command per
# instruction. Split every multi-wait instruction into N-1 preceding
# single-wait EventSemaphore instructions on the same engine.


def _legalize_sync_waits(bir_json: bytes) -> bytes:
    m = _json.loads(bir_json)
    ctr = 0
    for fn in m["functions"]:
        for bb in fn["blocks"]:
            out = []
            for ins in bb["instructions"]:
                si = ins.get("sync_info")
                waits = si.get("on_wait", []) if si else []
                if len(waits) > 1:
                    for w in waits[:-1]:
                        ctr += 1
                        out.append(
                            {
                                "debug": ins.get("debug", 0),
                                "engine": ins["engine"],
                                "ins": [],
                                "outs": [],
                                "name": f"evw-split-{ctr}",
                                "opcode": "EventSemaphore",
                                "sync_info": {"on_update": [], "on_wait": [w]},
                            }
                        )
                    si["on_wait"] = [waits[-1]]
                out.append(ins)
            bb["instructions"] = out
    return _json.dumps(m).encode()


_fixup_installed = False


def _install_bir_fixup():
    global _fixup_installed
    if _fixup_installed:
        return
    _fixup_installed = True
    import concourse.bass_utils as _bu

    _orig = _bu.compile_bir_kernel

    def _patched(bir_json, tmpdir, neff_name="file.neff"):
        if isinstance(bir_json, str):
            bir_json = bir_json.encode()
        return _orig(_legalize_sync_waits(bir_json), tmpdir, neff_name)

    _bu.compile_bir_kernel = _patched
    try:
        import concourse.bass2jax as _b2j

        _b2j.compile_bir_kernel = _patched
    except ImportError:
        pass


_install_bir_fixup()

B, N, D, H = 8, 1024, 768, 12
HD = D // H            # 64
F3 = 3 * D             # 2304
NCORE = 8
P = 128
NCHUNK = N // P        # 8 token chunks
KD = D // P            # 6 d_in chunks
QH = 512               # q-half size
NPAIR = H // 2         # 6
VW = HD + 1            # 65 (V cols + denominator ones col)
VPAD = 66              # padded per-head V width (4-byte aligned bf16)

f32 = mybir.dt.float32
f32r = mybir.dt.float32r
bf16 = mybir.dt.bfloat16
FT = mybir.ActivationFunctionType
ALU = mybir.AluOpType


def build_attention_nc():
    nc = bass.Bass()
    x_d = nc.declare_dram_parameter("x", [N, D], f32, isOutput=False)
    w_d = nc.declare_dram_parameter("W_qkv", [D, F3], f32, isOutput=False)
    b_d = nc.declare_dram_parameter("b_qkv", [F3], f32, isOutput=False)
    o_d = nc.declare_dram_parameter("out", [N, D], f32, isOutput=True)

    with tile.TileContext(nc) as tc, contextlib.ExitStack() as ctx:
        singles = ctx.enter_context(tc.tile_pool(name="singles", bufs=1))
        xpool = ctx.enter_context(tc.tile_pool(name="xpool", bufs=NCHUNK))
        xtpool = ctx.enter_context(tc.tile_pool(name="xtpool", bufs=KD))
        wpool = ctx.enter_context(tc.tile_pool(name="wpool", bufs=KD))
        qkpool = ctx.enter_context(tc.tile_pool(name="qkpool", bufs=10))
        vpool = ctx.enter_context(tc.tile_pool(name="vpool", bufs=NCHUNK))
        exppool = ctx.enter_context(tc.tile_pool(name="exppool", bufs=3))
        recpool = ctx.enter_context(tc.tile_pool(name="recpool", bufs=4))

        # PSUM budget (8 banks): wk [128,512] x2 = 2; sc [128,2,512] x2 = 4;
        # av [128,4,66] x2 = 2.
        wkps = ctx.enter_context(tc.tile_pool(name="wkps", bufs=2, space="PSUM"))
        scps = ctx.enter_context(tc.tile_pool(name="scps", bufs=2, space="PSUM"))
        avps = ctx.enter_context(tc.tile_pool(name="avps", bufs=2, space="PSUM"))

        def wk_psum():
            return wkps.tile([P, QH], f32, tag="wk", name="wktile")

        # ------------- constants -------------------------------------------
        ident = singles.tile([P, P], f32)
        make_identity(nc, ident)  # gpsimd

        ident_r = singles.tile([P, P], f32r)
        nc.vector.tensor_copy(out=ident_r, in_=ident)

        ones_f32 = singles.tile([P, 1], f32)
        nc.vector.memset(ones_f32, 1.0)
        ones_row_st = singles.tile([1, P], f32)
        nc.vector.memset(ones_row_st, 1.0)
        ones_row = singles.tile([1, P], f32r)
        nc.vector.tensor_copy(out=ones_row, in_=ones_row_st)

        # dummy exp to trigger the ACT table load early
        actwarm = singles.tile([1, 2], f32)
        nc.vector.memset(actwarm, 0.0)
        nc.scalar.activation(actwarm, actwarm, FT.Exp)

        b_sb = singles.tile([P, F3 // P], f32)
        nc.sync.dma_start(out=b_sb, in_=b_d[:].rearrange("(t p) -> p t", p=P))

        bv_st = singles.tile([1, D], f32)
        nc.sync.dma_start(out=bv_st, in_=b_d[2 * D : 3 * D][None, :])
        bv_sb = singles.tile([1, D], f32r)
        nc.vector.tensor_copy(out=bv_sb, in_=bv_st)

        # ------------- input DMAs (priority order) -------------------------
        x_sb = []
        for c in range(NCHUNK):
            t = xpool.tile([P, D], f32r, tag="x", name=f"x{c}")
            x_sb.append(t)

        def dma_x(c):
            nc.sync.dma_start(
                out=x_sb[c], in_=x_d[c * P : (c + 1) * P, :].bitcast(f32r)
            )

        w_sb = [wpool.tile([P, F3], f32r, tag="w", name=f"w{k}") for k in range(KD)]

        def dma_w_cols(f0, fw):
            for k in range(KD):
                nc.sync.dma_start(
                    out=w_sb[k][:, f0 : f0 + fw],
                    in_=w_d[k * P : (k + 1) * P, f0 : f0 + fw].bitcast(f32r),
                )

        for c in range(4):
            dma_x(c)
        dma_w_cols(0 * P, P)          # pair-0 Q cols
        dma_w_cols(6 * P, P)          # pair-0 K cols
        dma_w_cols(2 * D, 2 * P)      # V strip 0 (heads 0-3)
        for c in range(4, NCHUNK):
            dma_x(c)
        dma_w_cols(1 * P, P)          # pair-1 Q
        dma_w_cols(7 * P, P)          # pair-1 K
        dma_w_cols(2 * D + 2 * P, 2 * P)   # V strip 1 (heads 4-7)
        dma_w_cols(2 * P, P)
        dma_w_cols(8 * P, P)
        dma_w_cols(2 * D + 4 * P, 2 * P)   # V strip 2 (heads 8-11)
        for p in range(3, NPAIR):
            dma_w_cols(p * P, P)
            dma_w_cols((6 + p) * P, P)

        # ------------- x^T (PE transposes, batched copies) ------------------
        xt = [xtpool.tile([P, N], f32r, tag="xt", name=f"xt{k}") for k in range(KD)]

        def transpose_chunks(c0, nb=2):
            # transpose x chunks c0..c0+nb into xt[k][:, c0*P:(c0+nb)*P]
            for k in range(KD):
                ps = wk_psum()[:, 0 : nb * P]
                for j in range(nb):
                    nc.tensor.transpose(
                        ps[:, j * P : (j + 1) * P].bitcast(f32r),
                        x_sb[c0 + j][:, k * P : (k + 1) * P],
                        ident_r,
                    )
                nc.vector.tensor_copy(
                    out=xt[k][:, c0 * P : (c0 + nb) * P], in_=ps.bitcast(f32r)
                )

        # broadcast b_v across partitions once: bvb[p, f] = b_v[f]
        bvb = singles.tile([P, D], f32)
        for f0 in range(0, D, 256):
            ps = wk_psum()[:, 0:256]
            nc.tensor.matmul(
                ps, ones_row, bv_sb[:, f0 : f0 + 256], start=True, stop=True
            )
            nc.vector.tensor_copy(out=bvb[:, f0 : f0 + 256], in_=ps)

        # ------------- qk tiles ---------------------------------------------
        # qk[(f, half)]: [128, 512] bf16; partitions = features f*128..+128,
        # cols = tokens half*512..+512. f 0..5 = Q blocks, 6..11 = K blocks.
        qk_tiles = {}

        def get_qk(f, half):
            key = (f, half)
            if key not in qk_tiles:
                qk_tiles[key] = qkpool.tile(
                    [P, QH], bf16, tag="qk", name=f"qk{f}_{half}"
                )
            return qk_tiles[key]

        def make_qk(f, half, n0=0, nw=QH):
            # produce token-cols [n0, n0+nw) of tile (f, half); nw >= 256
            t = get_qk(f, half)
            ps = wk_psum()[:, 0:nw]
            for k in range(KD):
                nc.tensor.matmul(
                    ps,
                    w_sb[k][:, f * P : (f + 1) * P],
                    xt[k][:, half * QH + n0 : half * QH + n0 + nw],
                    start=(k == 0),
                    stop=(k == KD - 1),
                )
            nc.vector.tensor_scalar_add(
                t[:, n0 : n0 + nw], ps, b_sb[:, f : f + 1]
            )
            return t

        # ------------- V tiles ----------------------------------------------
        # v[c]: [128, 12, 66] bf16; [:, h, 0:64] = V for head h, [:, h, 64] = 1
        v_sb = []
        for c in range(NCHUNK):
            t = vpool.tile([P, H, VPAD], bf16, tag="v", name=f"v{c}")
            v_sb.append(t)

        def make_v(c, s):
            # strip s covers heads 4s..4s+4 (f-cols 2D + s*256 ..+256)
            if s == 0:
                nc.vector.tensor_copy(
                    out=v_sb[c][:, :, HD : HD + 1],
                    in_=ones_f32[:, 0:1, None].to_broadcast([P, H, 1]),
                )
            f0 = s * 256
            ps = wk_psum()[:, 0:256]
            for k in range(KD):
                nc.tensor.matmul(
                    ps,
                    xt[k][:, c * P : (c + 1) * P],
                    w_sb[k][:, 2 * D + f0 : 2 * D + f0 + 256],
                    start=(k == 0),
                    stop=(k == KD - 1),
                )
            nc.vector.tensor_tensor(
                v_sb[c][:, 4 * s : 4 * s + 4, 0:HD],
                ps.rearrange("p (h d) -> p h d", d=HD),
                bvb[:, f0 : f0 + 256].rearrange("p (h d) -> p h d", d=HD),
                ALU.add,
            )

        # ------------- bootstrap: transposes + first tiles ------------------
        transpose_chunks(0)
        transpose_chunks(2)
        make_qk(0, 0)                   # qt(pair0, qh0)
        make_qk(6, 0, 0, 256)           # kt(pair0) tokens 0:256
        make_qk(6, 0, 256, 256)         # kt(pair0) tokens 256:512

        onat = singles.tile([P, NCHUNK, D], f32)

        # JIT work queue: list of thunks, two popped per key-chunk iteration
        jit_q = []

        def run_jit(n):
            for _ in range(n):
                if jit_q:
                    jit_q.pop(0)()

        # pair 0 qh0 extra work: V strip0 chunks are emitted inline (AV needs
        # them); remaining bootstrap goes on the jit queue. Emission order is
        # tuned against the DMA arrival order so the shared wk PSUM ring
        # doesn't serialize early V production behind the x4-7 transposes.
        jit_q.append(lambda: None)                        # kc0
        jit_q.append(lambda: transpose_chunks(4))         # kc1
        jit_q.append(lambda: make_qk(6, 1, 0, 256))       # kc2
        jit_q.append(lambda: transpose_chunks(6))         # kc3
        jit_q.append(lambda: make_qk(6, 1, 256, 256))     # kc4
        jit_q.append(lambda: make_qk(0, 1))               # kc5

        # schedule of deferred production work, per (pair, qh):
        #   pair p qh0 -> kt(pair p+1) halves; pair p qh1 -> qt(p+1) halves
        #   V strips: strip1 over p0qh1+p1qh0, strip2 over p1qh1..p2qh1
        def sched(p, qh):
            w = []
            if p + 1 < NPAIR:
                if qh == 0:
                    w.append(lambda: make_qk(6 + p + 1, 0))
                    w.append(lambda: make_qk(6 + p + 1, 1))
                else:
                    w.append(lambda: make_qk(p + 1, 0))
                    w.append(lambda: make_qk(p + 1, 1))
            if (p, qh) == (0, 1):
                for c in range(4):
                    w.append(lambda c=c: make_v(c, 1))
            elif (p, qh) == (1, 0):
                for c in range(4, NCHUNK):
                    w.append(lambda c=c: make_v(c, 1))
            elif (p, qh) == (1, 1):
                for c in range(4):
                    w.append(lambda c=c: make_v(c, 2))
            elif (p, qh) == (2, 0):
                for c in range(4, NCHUNK):
                    w.append(lambda c=c: make_v(c, 2))
            return w

        # ------------- attention pair loop ----------------------------------
        pending = []  # deferred normalize work: (p, qh, av)

        def normalize(p, qh, av):
            # rc = 1/denominator, onat = av * rc
            for hi in range(2):
                h = 2 * p + hi
                rc = recpool.tile([P, 4], f32, tag="rec", name="rc")
                nc.vector.reciprocal(out=rc, in_=av[hi][:, :, HD])
                nc.vector.tensor_tensor(
                    onat[:, qh * 4 : (qh + 1) * 4, h * HD : (h + 1) * HD],
                    av[hi][:, :, 0:HD],
                    rc[:, :, None].to_broadcast([P, 4, HD]),
                    ALU.mult,
                )
            if p == NPAIR - 2 and qh == 1:
                # heads 0-9 of chunks 4-7 are final: DMA them now so only
                # the last pair's 128 columns remain for the tail
                for c in range(4, NCHUNK):
                    nc.sync.dma_start(
                        out=o_d[c * P : (c + 1) * P, 0 : 5 * P],
                        in_=onat[:, c, 0 : 5 * P],
                    )
            if p == NPAIR - 1:
                if qh == 0:
                    for c in range(4):
                        nc.sync.dma_start(
                            out=o_d[c * P : (c + 1) * P, :], in_=onat[:, c, :]
                        )
                else:
                    for c in range(4, NCHUNK):
                        nc.sync.dma_start(
                            out=o_d[c * P : (c + 1) * P, 5 * P : D],
                            in_=onat[:, c, 5 * P : D],
                        )

        for p in range(NPAIR):
            qt = [get_qk(p, 0), get_qk(p, 1)]
            kt = [get_qk(6 + p, 0), get_qk(6 + p, 1)]

            for qh in range(2):
                jit_q.extend(sched(p, qh))
                av = None
                for kc in range(NCHUNK):
                    kth = kt[kc // 4]
                    kcol = (kc % 4) * P
                    sc = scps.tile([P, 2, QH], f32, tag="sc", name="sc")
                    for hi in range(2):
                        nc.tensor.matmul(
                            sc[:, hi, :],
                            kth[64 * hi : 64 * hi + 64, kcol : kcol + P],
                            qt[qh][64 * hi : 64 * hi + 64, :],
                            start=True,
                            stop=True,
                            tile_position=(64 * hi, 0),
                        )
                    if p == 0 and qh == 0:
                        make_v(kc, 0)  # strip 0 JIT (AV below needs it)
                        run_jit(1)
                    else:
                        run_jit(2)
                    ex = exppool.tile([P, 2, QH], bf16, tag="exp", name="ex")
                    nc.scalar.activation(ex, sc, FT.Exp, scale=0.125)
                    if kc == 0:
                        # flush previous q-half's normalize after this
                        # half's first exp so ACT isn't gated on it
                        while pending:
                            normalize(*pending.pop(0))
                        av = [
                            avps.tile([P, 4, VPAD], f32, tag="av", name=f"av{hi}")
                            for hi in range(2)
                        ]
                    for hi in range(2):
                        for qc in range(4):
                            # one bank-clearing start per av bank: later
                            # first-writes overwrite per-element (has_written
                            # cleared), later kc's accumulate
                            nc.tensor.matmul(
                                av[hi][:, qc, 0:VW],
                                ex[:, hi, qc * P : (qc + 1) * P],
                                v_sb[kc][:, 2 * p + hi, 0:VW],
                                start=(kc == 0 and qc == 0),
                                stop=(kc == NCHUNK - 1 and qc == 3),
                                skip_group_check=True,
                            )
                pending.append((p, qh, av))
        while pending:
            normalize(*pending.pop(0))

    return nc


def kernel(x: np.ndarray, W_qkv: np.ndarray, b_qkv: np.ndarray) -> np.ndarray:
    nc = build_attention_nc()
    in_maps = [
        {
            "x": np.ascontiguousarray(x[c], dtype=np.float32),
            "W_qkv": np.ascontiguousarray(W_qkv, dtype=np.float32),
            "b_qkv": np.ascontiguousarray(b_qkv, dtype=np.float32),
        }
        for c in range(NCORE)
    ]
    res = run_bass_kernel_spmd(nc, in_maps, core_ids=list(range(NCORE)))
    return np.stack([res.results[c]["out"] for c in range(NCORE)], axis=0)


# revision 6
# speedup vs baseline: 1.0674x; 1.0345x over previous
"""Multi-head self-attention Trainium2 kernel (8 NeuronCores, batch-parallel).

Reference: qkv = x @ W_qkv + b; 12-head scaled-dot-product attention; concat.
Shapes: x[8,1024,768], W_qkv[768,2304], b_qkv[2304] -> out[8,1024,768].
Sharding: one batch element per core; W/b replicated to all cores.

Per-core dataflow:
  x --PE transpose--> xT[768,1024] (f32r), copies batched 4 chunks at a time
  qk tiles (bf16): per (f-block, token-half) [128,512] = W-block(lhsT) @ xT
    produced in N>=256 slices; Q/K biases added on the PSUM->SBUF copy
  V[128,12,66] bf16 per token chunk (strips of 4 heads; col 64 = ones)
  per (pair p, q-half qh), per key-chunk kc:
    scT[128,2,512] = K-slice(lhsT) @ Q-half  (2 row-tiled MMs, one per head)
    ex[128,2,512] bf16 = ACT Exp(0.125 * scT)   (scale folded into ACT)
    av[q=128,65] += ex-chunk(lhsT) @ [V_h|1]  bf16 N=65 MMs, accumulated
      over kc; av already in [q, feature] orientation, col 64 = denominator
  normalize: rc = 1/av[:,:,64] (DVE), onat[:, c, h*64:...] = av * rc
  out DMA per chunk once the last pair finishes its q-half.

Scheduling: W is DMA'd in priority order (pair-0 Q/K columns, V strip 0,
then later pairs); QK-tile and V-strip production is spread across the
pair loop just-in-time so PE work per key-chunk stays balanced against
the ACT exp stream (ACT is the co-bottleneck at ~1038ns per key-chunk).
"""

import contextlib
import json as _json

import numpy as np

import concourse.bass as bass
import concourse.mybir as mybir
import concourse.tile as tile
from concourse.bass_utils import run_bass_kernel_spmd
from concourse.masks import make_identity

# --- BIR sync-wait legalization ------------------------------------------
# walrus's codegen in this toolchain accepts only one sync-wait command per
# instruction. Split every multi-wait instruction into N-1 preceding
# single-wait EventSemaphore instructions on the same engine.


def _legalize_sync_waits(bir_json: bytes) -> bytes:
    m = _json.loads(bir_json)
    ctr = 0
    for fn in m["functions"]:
        for bb in fn["blocks"]:
            out = []
            for ins in bb["instructions"]:
                si = ins.get("sync_info")
                waits = si.get("on_wait", []) if si else []
                if len(waits) > 1:
                    for w in waits[:-1]:
                        ctr += 1
                        out.append(
                            {
                                "debug": ins.get("debug", 0),
                                "engine": ins["engine"],
                                "ins": [],
                                "outs": [],
                                "name": f"evw-split-{ctr}",
                                "opcode": "EventSemaphore",
                                "sync_info": {"on_update": [], "on_wait": [w]},
                            }
                        )
                    si["on_wait"] = [waits[-1]]
                out.append(ins)
            bb["instructions"] = out
    return _json.dumps(m).encode()


_fixup_installed = False


def _install_bir_fixup():
    global _fixup_installed
    if _fixup_installed:
        return
    _fixup_installed = True
    import concourse.bass_utils as _bu

    _orig = _bu.compile_bir_kernel

    def _patched(bir_json, tmpdir, neff_name="file.neff"):
        if isinstance(bir_json, str):
            bir_json = bir_json.encode()
        return _orig(_legalize_sync_waits(bir_json), tmpdir, neff_name)

    _bu.compile_bir_kernel = _patched
    try:
        import concourse.bass2jax as _b2j

        _b2j.compile_bir_kernel = _patched
    except ImportError:
        pass


_install_bir_fixup()

B, N, D, H = 8, 1024, 768, 12
HD = D // H            # 64
F3 = 3 * D             # 2304
NCORE = 8
P = 128
NCHUNK = N // P        # 8 token chunks
KD = D // P            # 6 d_in chunks
QH = 512               # q-half size
NPAIR = H // 2         # 6
VW = HD + 1            # 65 (V cols + denominator ones col)
VPAD = 66              # padded per-head V width (4-byte aligned bf16)

f32 = mybir.dt.float32
f32r = mybir.dt.float32r
bf16 = mybir.dt.bfloat16
FT = mybir.ActivationFunctionType
ALU = mybir.AluOpType


def build_attention_nc():
    nc = bass.Bass()
    x_d = nc.declare_dram_parameter("x", [N, D], f32, isOutput=False)
    w_d = nc.declare_dram_parameter("W_qkv", [D, F3], f32, isOutput=False)
    b_d = nc.declare_dram_parameter("b_qkv", [F3], f32, isOutput=False)
    o_d = nc.declare_dram_parameter("out", [N, D], f32, isOutput=True)

    with tile.TileContext(nc) as tc, contextlib.ExitStack() as ctx:
        singles = ctx.enter_context(tc.tile_pool(name="singles", bufs=1))
        xpool = ctx.enter_context(tc.tile_pool(name="xpool", bufs=NCHUNK))
        xtpool = ctx.enter_context(tc.tile_pool(name="xtpool", bufs=KD))
        wpool = ctx.enter_context(tc.tile_pool(name="wpool", bufs=KD))
        qkpool = ctx.enter_context(tc.tile_pool(name="qkpool", bufs=10))
        vpool = ctx.enter_context(tc.tile_pool(name="vpool", bufs=NCHUNK))
        exppool = ctx.enter_context(tc.tile_pool(name="exppool", bufs=3))
        recpool = ctx.enter_context(tc.tile_pool(name="recpool", bufs=4))

        # PSUM budget (8 banks): wk [128,512] x2 = 2; sc [128,2,512] x2 = 4;
        # av [128,4,66] x2 = 2.
        wkps = ctx.enter_context(tc.tile_pool(name="wkps", bufs=2, space="PSUM"))
        scps = ctx.enter_context(tc.tile_pool(name="scps", bufs=2, space="PSUM"))
        avps = ctx.enter_context(tc.tile_pool(name="avps", bufs=2, space="PSUM"))

        def wk_psum():
            return wkps.tile([P, QH], f32, tag="wk", name="wktile")

        # ------------- constants -------------------------------------------
        ident = singles.tile([P, P], f32)
        make_identity(nc, ident)  # gpsimd

        ident_r = singles.tile([P, P], f32r)
        nc.vector.tensor_copy(out=ident_r, in_=ident)

        ones_f32 = singles.tile([P, 1], f32)
        nc.vector.memset(ones_f32, 1.0)
        ones_row_st = singles.tile([1, P], f32)
        nc.vector.memset(ones_row_st, 1.0)
        ones_row = singles.tile([1, P], f32r)
        nc.vector.tensor_copy(out=ones_row, in_=ones_row_st)

        # dummy exp to trigger the ACT table load early
        actwarm = singles.tile([1, 2], f32)
        nc.vector.memset(actwarm, 0.0)
        nc.scalar.activation(actwarm, actwarm, FT.Exp)

        bv_st = singles.tile([1, D], f32)
        nc.sync.dma_start(out=bv_st, in_=b_d[2 * D : 3 * D][None, :])
        bv_sb = singles.tile([1, D], f32r)
        nc.vector.tensor_copy(out=bv_sb, in_=bv_st)

        # ------------- input DMAs (batched, priority order) ----------------
        # HWDGE charges a flat ~625ns per DMA instruction, serialized — so
        # batch: one DMA per W column block covering all 6 k-chunks, and
        # 2-chunk x DMAs.
        x_big = singles.tile([P, NCHUNK, D], f32r)
        x_sb = [x_big[:, c, :] for c in range(NCHUNK)]

        def dma_x2(c0):
            nc.sync.dma_start(
                out=x_big[:, c0 : c0 + 2, :],
                in_=x_d[c0 * P : (c0 + 2) * P, :]
                .bitcast(f32r)
                .rearrange("(c p) d -> p c d", p=P),
            )

        w_big = singles.tile([P, KD, F3], f32r)
        w_sb = [w_big[:, k, :] for k in range(KD)]

        def dma_w_cols(f0, fw):
            nc.sync.dma_start(
                out=w_big[:, :, f0 : f0 + fw],
                in_=w_d[:, f0 : f0 + fw]
                .bitcast(f32r)
                .rearrange("(k p) f -> p k f", p=P),
            )

        dma_x2(0)
        dma_x2(2)
        dma_w_cols(0 * P, P)          # pair-0 Q cols
        dma_w_cols(6 * P, P)          # pair-0 K cols
        dma_w_cols(2 * D, 2 * P)      # V strip 0 (heads 0-3)
        b_sb = singles.tile([P, F3 // P], f32)
        nc.sync.dma_start(out=b_sb, in_=b_d[:].rearrange("(t p) -> p t", p=P))
        dma_x2(4)
        dma_x2(6)
        dma_w_cols(1 * P, P)          # pair-1 Q
        dma_w_cols(7 * P, P)          # pair-1 K
        dma_w_cols(2 * D + 2 * P, 2 * P)   # V strip 1 (heads 4-7)
        dma_w_cols(2 * P, P)
        dma_w_cols(8 * P, P)
        dma_w_cols(2 * D + 4 * P, 2 * P)   # V strip 2 (heads 8-11)
        for p in range(3, NPAIR):
            dma_w_cols(p * P, P)
            dma_w_cols((6 + p) * P, P)

        # ------------- x^T (PE transposes, batched copies) ------------------
        xt = [xtpool.tile([P, N], f32r, tag="xt", name=f"xt{k}") for k in range(KD)]

        def transpose_chunks(c0, nb=2):
            # transpose x chunks c0..c0+nb into xt[k][:, c0*P:(c0+nb)*P]
            for k in range(KD):
                ps = wk_psum()[:, 0 : nb * P]
                for j in range(nb):
                    nc.tensor.transpose(
                        ps[:, j * P : (j + 1) * P].bitcast(f32r),
                        x_sb[c0 + j][:, k * P : (k + 1) * P],
                        ident_r,
                    )
                nc.vector.tensor_copy(
                    out=xt[k][:, c0 * P : (c0 + nb) * P], in_=ps.bitcast(f32r)
                )

        # broadcast b_v across partitions once: bvb[p, f] = b_v[f]
        bvb = singles.tile([P, D], f32)
        for f0 in range(0, D, 256):
            ps = wk_psum()[:, 0:256]
            nc.tensor.matmul(
                ps, ones_row, bv_sb[:, f0 : f0 + 256], start=True, stop=True
            )
            nc.vector.tensor_copy(out=bvb[:, f0 : f0 + 256], in_=ps)

        # ------------- qk tiles ---------------------------------------------
        # qk[(f, half)]: [128, 512] bf16; partitions = features f*128..+128,
        # cols = tokens half*512..+512. f 0..5 = Q blocks, 6..11 = K blocks.
        qk_tiles = {}

        def get_qk(f, half):
            key = (f, half)
            if key not in qk_tiles:
                qk_tiles[key] = qkpool.tile(
                    [P, QH], bf16, tag="qk", name=f"qk{f}_{half}"
                )
            return qk_tiles[key]

        def make_qk(f, half, n0=0, nw=QH):
            # produce token-cols [n0, n0+nw) of tile (f, half); nw >= 256
            t = get_qk(f, half)
            ps = wk_psum()[:, 0:nw]
            for k in range(KD):
                nc.tensor.matmul(
                    ps,
                    w_sb[k][:, f * P : (f + 1) * P],
                    xt[k][:, half * QH + n0 : half * QH + n0 + nw],
                    start=(k == 0),
                    stop=(k == KD - 1),
                )
            nc.vector.tensor_scalar_add(
                t[:, n0 : n0 + nw], ps, b_sb[:, f : f + 1]
            )
            return t

        # ------------- V tiles ----------------------------------------------
        # v[c]: [128, 12, 66] bf16; [:, h, 0:64] = V for head h, [:, h, 64] = 1
        v_sb = []
        for c in range(NCHUNK):
            t = vpool.tile([P, H, VPAD], bf16, tag="v", name=f"v{c}")
            v_sb.append(t)

        def make_v(c, s):
            # strip s covers heads 4s..4s+4 (f-cols 2D + s*256 ..+256)
            if s == 0:
                nc.vector.tensor_copy(
                    out=v_sb[c][:, :, HD : HD + 1],
                    in_=ones_f32[:, 0:1, None].to_broadcast([P, H, 1]),
                )
            f0 = s * 256
            ps = wk_psum()[:, 0:256]
            for k in range(KD):
                nc.tensor.matmul(
                    ps,
                    xt[k][:, c * P : (c + 1) * P],
                    w_sb[k][:, 2 * D + f0 : 2 * D + f0 + 256],
                    start=(k == 0),
                    stop=(k == KD - 1),
                )
            nc.vector.tensor_tensor(
                v_sb[c][:, 4 * s : 4 * s + 4, 0:HD],
                ps.rearrange("p (h d) -> p h d", d=HD),
                bvb[:, f0 : f0 + 256].rearrange("p (h d) -> p h d", d=HD),
                ALU.add,
            )

        # ------------- bootstrap: transposes + first tiles ------------------
        transpose_chunks(0)
        transpose_chunks(2)
        make_qk(0, 0)                   # qt(pair0, qh0)
        make_qk(6, 0, 0, 256)           # kt(pair0) tokens 0:256
        make_qk(6, 0, 256, 256)         # kt(pair0) tokens 256:512

        onat = singles.tile([P, NCHUNK, D], f32)

        # JIT work queue: list of thunks, two popped per key-chunk iteration
        jit_q = []

        def run_jit(n):
            for _ in range(n):
                if jit_q:
                    jit_q.pop(0)()

        # pair 0 qh0 extra work: V strip0 chunks are emitted inline (AV needs
        # them); remaining bootstrap goes on the jit queue. Emission order is
        # tuned against the DMA arrival order so the shared wk PSUM ring
        # doesn't serialize early V production behind the x4-7 transposes.
        jit_q.append(lambda: None)                        # kc0
        jit_q.append(lambda: transpose_chunks(4))         # kc1
        jit_q.append(lambda: make_qk(6, 1, 0, 256))       # kc2
        jit_q.append(lambda: transpose_chunks(6))         # kc3
        jit_q.append(lambda: make_qk(6, 1, 256, 256))     # kc4
        jit_q.append(lambda: make_qk(0, 1))               # kc5

        # schedule of deferred production work, per (pair, qh):
        #   pair p qh0 -> kt(pair p+1) halves; pair p qh1 -> qt(p+1) halves
        #   V strips: strip1 over p0qh1+p1qh0, strip2 over p1qh1..p2qh1
        def sched(p, qh):
            w = []
            if p + 1 < NPAIR:
                if qh == 0:
                    w.append(lambda: make_qk(6 + p + 1, 0))
                    w.append(lambda: make_qk(6 + p + 1, 1))
                else:
                    w.append(lambda: make_qk(p + 1, 0))
                    w.append(lambda: make_qk(p + 1, 1))
            if (p, qh) == (0, 1):
                for c in range(4):
                    w.append(lambda c=c: make_v(c, 1))
            elif (p, qh) == (1, 0):
                for c in range(4, NCHUNK):
                    w.append(lambda c=c: make_v(c, 1))
            elif (p, qh) == (1, 1):
                for c in range(4):
                    w.append(lambda c=c: make_v(c, 2))
            elif (p, qh) == (2, 0):
                for c in range(4, NCHUNK):
                    w.append(lambda c=c: make_v(c, 2))
            return w

        # ------------- attention pair loop ----------------------------------
        def normalize(p, qh, av):
            # rc = 1/denominator, onat = av * rc
            for hi in range(2):
                h = 2 * p + hi
                rc = recpool.tile([P, 4], f32, tag="rec", name="rc")
                nc.vector.reciprocal(out=rc, in_=av[hi][:, :, HD])
                nc.vector.tensor_tensor(
                    onat[:, qh * 4 : (qh + 1) * 4, h * HD : (h + 1) * HD],
                    av[hi][:, :, 0:HD],
                    rc[:, :, None].to_broadcast([P, 4, HD]),
                    ALU.mult,
                )
            if p == NPAIR - 2 and qh == 1:
                # heads 0-9 of chunks 4-7 are final: DMA them now so only
                # the last pair's 128 columns remain for the tail
                nc.sync.dma_start(
                    out=o_d[4 * P : NCHUNK * P, 0 : 5 * P].rearrange(
                        "(c p) f -> p c f", p=P
                    ),
                    in_=onat[:, 4:NCHUNK, 0 : 5 * P],
                )
            if p == NPAIR - 1:
                if qh == 0:
                    nc.sync.dma_start(
                        out=o_d[0 : 4 * P, :].rearrange("(c p) f -> p c f", p=P),
                        in_=onat[:, 0:4, :],
                    )
                else:
                    nc.sync.dma_start(
                        out=o_d[4 * P : NCHUNK * P, 5 * P : D].rearrange(
                            "(c p) f -> p c f", p=P
                        ),
                        in_=onat[:, 4:NCHUNK, 5 * P : D],
                    )

        # software pipeline: AV for iteration i emitted during iteration i+1,
        # so the next sc MMs (and the exp they feed) aren't serialized behind
        # the AV tail at q-half boundaries.
        stream = [
            (p, qh, kc)
            for p in range(NPAIR)
            for qh in range(2)
            for kc in range(NCHUNK)
        ]
        av_cur = None
        deferred = None  # (p, qh, kc, av, ex) AV work from previous iteration

        def emit_av(p, qh, kc, av, ex):
            for hi in range(2):
                for qc in range(4):
                    # one bank-clearing start per av bank: later first-writes
                    # overwrite per-element (has_written cleared by the
                    # start), later kc's accumulate
                    nc.tensor.matmul(
                        av[hi][:, qc, 0:VW],
                        ex[:, hi, qc * P : (qc + 1) * P],
                        v_sb[kc][:, 2 * p + hi, 0:VW],
                        start=(kc == 0 and qc == 0),
                        stop=(kc == NCHUNK - 1 and qc == 3),
                        skip_group_check=True,
                    )

        for p, qh, kc in stream:
            if kc == 0:
                jit_q.extend(sched(p, qh))
            qt = get_qk(p, qh)
            kth = get_qk(6 + p, kc // 4)
            kcol = (kc % 4) * P
            sc = scps.tile([P, 2, QH], f32, tag="sc", name="sc")
            for hi in range(2):
                nc.tensor.matmul(
                    sc[:, hi, :],
                    kth[64 * hi : 64 * hi + 64, kcol : kcol + P],
                    qt[64 * hi : 64 * hi + 64, :],
                    start=True,
                    stop=True,
                    tile_position=(64 * hi, 0),
                )
            if p == 0 and qh == 0:
                make_v(kc, 0)  # strip 0 JIT (AV needs it next iteration)
                run_jit(1)
            else:
                run_jit(2)
            ex = exppool.tile([P, 2, QH], bf16, tag="exp", name="ex")
            nc.scalar.activation(ex, sc, FT.Exp, scale=0.125)
            if deferred is not None:
                emit_av(*deferred)
                dp, dqh, dkc, dav, _ = deferred
                if dkc == NCHUNK - 1:
                    normalize(dp, dqh, dav)
            if kc == 0:
                av_cur = [
                    avps.tile([P, 4, VPAD], f32, tag="av", name=f"av{hi}")
                    for hi in range(2)
                ]
            deferred = (p, qh, kc, av_cur, ex)
        emit_av(*deferred)
        normalize(NPAIR - 1, 1, av_cur)

    return nc


def kernel(x: np.ndarray, W_qkv: np.ndarray, b_qkv: np.ndarray) -> np.ndarray:
    nc = build_attention_nc()
    in_maps = [
        {
            "x": np.ascontiguousarray(x[c], dtype=np.float32),
            "W_qkv": np.ascontiguousarray(W_qkv, dtype=np.float32),
            "b_qkv": np.ascontiguousarray(b_qkv, dtype=np.float32),
        }
        for c in range(NCORE)
    ]
    res = run_bass_kernel_spmd(nc, in_maps, core_ids=list(range(NCORE)))
    return np.stack([res.results[c]["out"] for c in range(NCORE)], axis=0)


# revision 7
# speedup vs baseline: 1.0824x; 1.0141x over previous
"""Multi-head self-attention Trainium2 kernel (8 NeuronCores, batch-parallel).

Reference: qkv = x @ W_qkv + b; 12-head scaled-dot-product attention; concat.
Shapes: x[8,1024,768], W_qkv[768,2304], b_qkv[2304] -> out[8,1024,768].
Sharding: one batch element per core; W/b replicated to all cores.

Per-core dataflow:
  x --PE transpose--> xT[768,1024] (f32r), copies batched 4 chunks at a time
  qk tiles (bf16): per (f-block, token-half) [128,512] = W-block(lhsT) @ xT
    produced in N>=256 slices; Q/K biases added on the PSUM->SBUF copy
  V[128,12,66] bf16 per token chunk (strips of 4 heads; col 64 = ones)
  per (pair p, q-half qh), per key-chunk kc:
    scT[128,2,512] = K-slice(lhsT) @ Q-half  (2 row-tiled MMs, one per head)
    ex[128,2,512] bf16 = ACT Exp(0.125 * scT)   (scale folded into ACT)
    av[q=128,65] += ex-chunk(lhsT) @ [V_h|1]  bf16 N=65 MMs, accumulated
      over kc; av already in [q, feature] orientation, col 64 = denominator
  normalize: rc = 1/av[:,:,64] (DVE), onat[:, c, h*64:...] = av * rc
  out DMA per chunk once the last pair finishes its q-half.

Scheduling: W is DMA'd in priority order (pair-0 Q/K columns, V strip 0,
then later pairs); QK-tile and V-strip production is spread across the
pair loop just-in-time so PE work per key-chunk stays balanced against
the ACT exp stream (ACT is the co-bottleneck at ~1038ns per key-chunk).
"""

import contextlib
import json as _json

import numpy as np

import concourse.bass as bass
import concourse.mybir as mybir
import concourse.tile as tile
from concourse.bass_utils import run_bass_kernel_spmd
from concourse.masks import make_identity

# --- BIR sync-wait legalization ------------------------------------------
# walrus's codegen in this toolchain accepts only one sync-wait command per
# instruction. Split every multi-wait instruction into N-1 preceding
# single-wait EventSemaphore instructions on the same engine.


def _legalize_sync_waits(bir_json: bytes) -> bytes:
    m = _json.loads(bir_json)
    ctr = 0
    for fn in m["functions"]:
        for bb in fn["blocks"]:
            out = []
            for ins in bb["instructions"]:
                si = ins.get("sync_info")
                waits = si.get("on_wait", []) if si else []
                if len(waits) > 1:
                    for w in waits[:-1]:
                        ctr += 1
                        out.append(
                            {
                                "debug": ins.get("debug", 0),
                                "engine": ins["engine"],
                                "ins": [],
                                "outs": [],
                                "name": f"evw-split-{ctr}",
                                "opcode": "EventSemaphore",
                                "sync_info": {"on_update": [], "on_wait": [w]},
                            }
                        )
                    si["on_wait"] = [waits[-1]]
                out.append(ins)
            bb["instructions"] = out
    return _json.dumps(m).encode()


_fixup_installed = False


def _install_bir_fixup():
    global _fixup_installed
    if _fixup_installed:
        return
    _fixup_installed = True
    import concourse.bass_utils as _bu

    _orig = _bu.compile_bir_kernel

    def _patched(bir_json, tmpdir, neff_name="file.neff"):
        if isinstance(bir_json, str):
            bir_json = bir_json.encode()
        return _orig(_legalize_sync_waits(bir_json), tmpdir, neff_name)

    _bu.compile_bir_kernel = _patched
    try:
        import concourse.bass2jax as _b2j

        _b2j.compile_bir_kernel = _patched
    except ImportError:
        pass


_install_bir_fixup()

B, N, D, H = 8, 1024, 768, 12
HD = D // H            # 64
F3 = 3 * D             # 2304
NCORE = 8
P = 128
NCHUNK = N // P        # 8 token chunks
KD = D // P            # 6 d_in chunks
QH = 512               # q-half size
NPAIR = H // 2         # 6
VW = HD + 1            # 65 (V cols + denominator ones col)
VPAD = 66              # padded per-head V width (4-byte aligned bf16)

f32 = mybir.dt.float32
f32r = mybir.dt.float32r
bf16 = mybir.dt.bfloat16
FT = mybir.ActivationFunctionType
ALU = mybir.AluOpType


def build_attention_nc():
    nc = bass.Bass()
    x_d = nc.declare_dram_parameter("x", [N, D], f32, isOutput=False)
    w_d = nc.declare_dram_parameter("W_qkv", [D, F3], f32, isOutput=False)
    b_d = nc.declare_dram_parameter("b_qkv", [F3], f32, isOutput=False)
    o_d = nc.declare_dram_parameter("out", [N, D], f32, isOutput=True)

    with tile.TileContext(nc) as tc, contextlib.ExitStack() as ctx:
        singles = ctx.enter_context(tc.tile_pool(name="singles", bufs=1))
        xpool = ctx.enter_context(tc.tile_pool(name="xpool", bufs=NCHUNK))
        xtpool = ctx.enter_context(tc.tile_pool(name="xtpool", bufs=KD))
        wpool = ctx.enter_context(tc.tile_pool(name="wpool", bufs=KD))
        qkpool = ctx.enter_context(tc.tile_pool(name="qkpool", bufs=10))
        vpool = ctx.enter_context(tc.tile_pool(name="vpool", bufs=NCHUNK))
        exppool = ctx.enter_context(tc.tile_pool(name="exppool", bufs=4))
        recpool = ctx.enter_context(tc.tile_pool(name="recpool", bufs=4))

        # PSUM budget (8 banks): wk [128,512] x2 = 2; sc [128,2,512] x2 = 4;
        # av [128,4,66] x2 = 2.
        wkps = ctx.enter_context(tc.tile_pool(name="wkps", bufs=2, space="PSUM"))
        scps = ctx.enter_context(tc.tile_pool(name="scps", bufs=2, space="PSUM"))
        avps = ctx.enter_context(tc.tile_pool(name="avps", bufs=2, space="PSUM"))

        def wk_psum():
            return wkps.tile([P, QH], f32, tag="wk", name="wktile")

        # ------------- constants -------------------------------------------
        ident = singles.tile([P, P], f32)
        make_identity(nc, ident)  # gpsimd

        ident_r = singles.tile([P, P], f32r)
        nc.vector.tensor_copy(out=ident_r, in_=ident)

        ones_f32 = singles.tile([P, 1], f32)
        nc.vector.memset(ones_f32, 1.0)
        ones_row_st = singles.tile([1, P], f32)
        nc.vector.memset(ones_row_st, 1.0)
        ones_row = singles.tile([1, P], f32r)
        nc.vector.tensor_copy(out=ones_row, in_=ones_row_st)

        # dummy exp to trigger the ACT table load early
        actwarm = singles.tile([1, 2], f32)
        nc.vector.memset(actwarm, 0.0)
        nc.scalar.activation(actwarm, actwarm, FT.Exp)

        bv_st = singles.tile([1, D], f32)
        nc.sync.dma_start(out=bv_st, in_=b_d[2 * D : 3 * D][None, :])
        bv_sb = singles.tile([1, D], f32r)
        nc.vector.tensor_copy(out=bv_sb, in_=bv_st)

        # ------------- input DMAs (batched, priority order) ----------------
        # HWDGE charges a flat ~625ns per DMA instruction, serialized — so
        # batch: one DMA per W column block covering all 6 k-chunks, and
        # 2-chunk x DMAs.
        x_big = singles.tile([P, NCHUNK, D], f32r)
        x_sb = [x_big[:, c, :] for c in range(NCHUNK)]

        def dma_x2(c0):
            nc.sync.dma_start(
                out=x_big[:, c0 : c0 + 2, :],
                in_=x_d[c0 * P : (c0 + 2) * P, :]
                .bitcast(f32r)
                .rearrange("(c p) d -> p c d", p=P),
            )

        w_big = singles.tile([P, KD, F3], f32r)
        w_sb = [w_big[:, k, :] for k in range(KD)]

        def dma_w_cols(f0, fw):
            nc.sync.dma_start(
                out=w_big[:, :, f0 : f0 + fw],
                in_=w_d[:, f0 : f0 + fw]
                .bitcast(f32r)
                .rearrange("(k p) f -> p k f", p=P),
            )

        dma_x2(0)
        dma_x2(2)
        dma_w_cols(0 * P, P)          # pair-0 Q cols
        b_sb = singles.tile([P, 2 * KD], f32)  # Q/K biases only; V uses bv
        nc.sync.dma_start(
            out=b_sb, in_=b_d[0 : 2 * D].rearrange("(t p) -> p t", p=P)
        )
        dma_w_cols(6 * P, P)          # pair-0 K cols
        dma_w_cols(2 * D, 2 * P)      # V strip 0 (heads 0-3)
        dma_x2(4)
        dma_x2(6)
        dma_w_cols(1 * P, P)          # pair-1 Q
        dma_w_cols(7 * P, P)          # pair-1 K
        dma_w_cols(2 * D + 2 * P, 2 * P)   # V strip 1 (heads 4-7)
        dma_w_cols(2 * P, P)
        dma_w_cols(8 * P, P)
        dma_w_cols(2 * D + 4 * P, 2 * P)   # V strip 2 (heads 8-11)
        for p in range(3, NPAIR):
            dma_w_cols(p * P, P)
            dma_w_cols((6 + p) * P, P)

        # ------------- x^T (PE transposes, batched copies) ------------------
        xt = [xtpool.tile([P, N], f32r, tag="xt", name=f"xt{k}") for k in range(KD)]

        def transpose_chunks(c0, nb=2):
            # transpose x chunks c0..c0+nb into xt[k][:, c0*P:(c0+nb)*P]
            for k in range(KD):
                ps = wk_psum()[:, 0 : nb * P]
                for j in range(nb):
                    nc.tensor.transpose(
                        ps[:, j * P : (j + 1) * P].bitcast(f32r),
                        x_sb[c0 + j][:, k * P : (k + 1) * P],
                        ident_r,
                    )
                nc.vector.tensor_copy(
                    out=xt[k][:, c0 * P : (c0 + nb) * P], in_=ps.bitcast(f32r)
                )

        # broadcast b_v across partitions once: bvb[p, f] = b_v[f]
        bvb = singles.tile([P, D], f32)
        for f0 in range(0, D, 256):
            ps = wk_psum()[:, 0:256]
            nc.tensor.matmul(
                ps, ones_row, bv_sb[:, f0 : f0 + 256], start=True, stop=True
            )
            nc.vector.tensor_copy(out=bvb[:, f0 : f0 + 256], in_=ps)

        # ------------- qk tiles ---------------------------------------------
        # qk[(f, half)]: [128, 512] bf16; partitions = features f*128..+128,
        # cols = tokens half*512..+512. f 0..5 = Q blocks, 6..11 = K blocks.
        qk_tiles = {}

        def get_qk(f, half):
            key = (f, half)
            if key not in qk_tiles:
                qk_tiles[key] = qkpool.tile(
                    [P, QH], bf16, tag="qk", name=f"qk{f}_{half}"
                )
            return qk_tiles[key]

        def make_qk(f, half, n0=0, nw=QH, ks=0, ke=KD, _ps=[None]):
            # produce token-cols [n0, n0+nw) of tile (f, half); nw >= 256.
            # ks/ke allow k-chunk-split emission (jit pacing); the PSUM tile
            # is carried across the split via _ps.
            t = get_qk(f, half)
            if ks == 0:
                _ps[0] = wk_psum()[:, 0:nw]
            ps = _ps[0]
            for k in range(ks, ke):
                nc.tensor.matmul(
                    ps,
                    w_sb[k][:, f * P : (f + 1) * P],
                    xt[k][:, half * QH + n0 : half * QH + n0 + nw],
                    start=(k == 0),
                    stop=(k == KD - 1),
                )
            if ke == KD:
                nc.vector.tensor_scalar_add(
                    t[:, n0 : n0 + nw], ps, b_sb[:, f : f + 1]
                )
            return t

        def qk_halves(f, half):
            # two pacing thunks producing tile (f, half) split by k-chunks
            return [
                lambda: make_qk(f, half, ks=0, ke=3),
                lambda: make_qk(f, half, ks=3, ke=KD),
            ]

        # ------------- V tiles ----------------------------------------------
        # v[c]: [128, 12, 66] bf16; [:, h, 0:64] = V for head h, [:, h, 64] = 1
        v_sb = []
        for c in range(NCHUNK):
            t = vpool.tile([P, H, VPAD], bf16, tag="v", name=f"v{c}")
            v_sb.append(t)

        def make_v(c, s):
            # strip s covers heads 4s..4s+4 (f-cols 2D + s*256 ..+256)
            if s == 0:
                nc.vector.tensor_copy(
                    out=v_sb[c][:, :, HD : HD + 1],
                    in_=ones_f32[:, 0:1, None].to_broadcast([P, H, 1]),
                )
            f0 = s * 256
            ps = wk_psum()[:, 0:256]
            for k in range(KD):
                nc.tensor.matmul(
                    ps,
                    xt[k][:, c * P : (c + 1) * P],
                    w_sb[k][:, 2 * D + f0 : 2 * D + f0 + 256],
                    start=(k == 0),
                    stop=(k == KD - 1),
                )
            nc.vector.tensor_tensor(
                v_sb[c][:, 4 * s : 4 * s + 4, 0:HD],
                ps.rearrange("p (h d) -> p h d", d=HD),
                bvb[:, f0 : f0 + 256].rearrange("p (h d) -> p h d", d=HD),
                ALU.add,
            )

        # ------------- bootstrap: transposes + first tiles ------------------
        transpose_chunks(0)
        transpose_chunks(2)
        make_qk(0, 0)                   # qt(pair0, qh0)
        make_qk(6, 0, 0, 256)           # kt(pair0) tokens 0:256

        onat = singles.tile([P, NCHUNK, D], f32)

        # JIT work queue: list of thunks, one popped per key-chunk iteration
        jit_q = []

        def run_jit(n):
            for _ in range(n):
                if jit_q:
                    jit_q.pop(0)()

        # pair 0 qh0 extra work: V strip0 chunks are emitted inline (AV needs
        # them); remaining bootstrap goes on the jit queue. Emission order is
        # tuned against the DMA arrival order so the shared wk PSUM ring
        # doesn't serialize early V production behind the x4-7 transposes,
        # and each tile completes at least one iteration before first read.
        jit_q.append(lambda: make_qk(6, 0, 256, 256))     # kc0
        jit_q.append(lambda: transpose_chunks(4))         # kc1
        jit_q.append(lambda: make_qk(6, 1, 0, 256))       # kc2
        jit_q.append(lambda: transpose_chunks(6))         # kc3
        jit_q.append(lambda: make_qk(6, 1, 256, 256))     # kc4
        jit_q.extend(qk_halves(0, 1))                     # kc5, kc6

        # schedule of deferred production work, per (pair, qh):
        #   pair p qh0 -> kt(pair p+1) halves; pair p qh1 -> qt(p+1) halves
        #   V strips: strip1 over p0qh1+p1qh0, strip2 over p1qh1..p2qh1
        def sched(p, qh):
            w = []
            if p + 1 < NPAIR:
                if qh == 0:
                    w += qk_halves(6 + p + 1, 0)
                    w += qk_halves(6 + p + 1, 1)
                else:
                    w += qk_halves(p + 1, 0)
                    w += qk_halves(p + 1, 1)
            if (p, qh) == (0, 1):
                for c in range(4):
                    w.append(lambda c=c: make_v(c, 1))
            elif (p, qh) == (1, 0):
                for c in range(4, NCHUNK):
                    w.append(lambda c=c: make_v(c, 1))
            elif (p, qh) == (1, 1):
                for c in range(4):
                    w.append(lambda c=c: make_v(c, 2))
            elif (p, qh) == (2, 0):
                for c in range(4, NCHUNK):
                    w.append(lambda c=c: make_v(c, 2))
            return w

        # ------------- attention pair loop ----------------------------------
        def normalize(p, qh, av):
            # rc = 1/denominator, onat = av * rc
            for hi in range(2):
                h = 2 * p + hi
                rc = recpool.tile([P, 4], f32, tag="rec", name="rc")
                nc.vector.reciprocal(out=rc, in_=av[hi][:, :, HD])
                nc.vector.tensor_tensor(
                    onat[:, qh * 4 : (qh + 1) * 4, h * HD : (h + 1) * HD],
                    av[hi][:, :, 0:HD],
                    rc[:, :, None].to_broadcast([P, 4, HD]),
                    ALU.mult,
                )
                if p == NPAIR - 1 and qh == 1:
                    # final-tail head: DMA its 64 columns immediately
                    nc.sync.dma_start(
                        out=o_d[4 * P : NCHUNK * P, h * HD : (h + 1) * HD]
                        .rearrange("(c p) f -> p c f", p=P),
                        in_=onat[:, 4:NCHUNK, h * HD : (h + 1) * HD],
                    )
            if p == NPAIR - 2 and qh == 1:
                # heads 0-9 of chunks 4-7 are final: DMA them now so only
                # the last pair's 128 columns remain for the tail
                nc.sync.dma_start(
                    out=o_d[4 * P : NCHUNK * P, 0 : 5 * P].rearrange(
                        "(c p) f -> p c f", p=P
                    ),
                    in_=onat[:, 4:NCHUNK, 0 : 5 * P],
                )
            if p == NPAIR - 1 and qh == 0:
                nc.sync.dma_start(
                    out=o_d[0 : 4 * P, :].rearrange("(c p) f -> p c f", p=P),
                    in_=onat[:, 0:4, :],
                )

        # software pipeline: AV for iteration i emitted during iteration i+1,
        # so the next sc MMs (and the exp they feed) aren't serialized behind
        # the AV tail at q-half boundaries.
        stream = [
            (p, qh, kc)
            for p in range(NPAIR)
            for qh in range(2)
            for kc in range(NCHUNK)
        ]
        av_cur = None
        deferred = None  # (p, qh, kc, av, ex) AV work from previous iteration

        def emit_av(p, qh, kc, av, ex):
            for hi in range(2):
                for qc in range(4):
                    # one bank-clearing start per av bank: later first-writes
                    # overwrite per-element (has_written cleared by the
                    # start), later kc's accumulate
                    nc.tensor.matmul(
                        av[hi][:, qc, 0:VW],
                        ex[:, hi, qc * P : (qc + 1) * P],
                        v_sb[kc][:, 2 * p + hi, 0:VW],
                        start=(kc == 0 and qc == 0),
                        stop=(kc == NCHUNK - 1 and qc == 3),
                        skip_group_check=True,
                    )

        for p, qh, kc in stream:
            if kc == 0:
                jit_q.extend(sched(p, qh))
            qt = get_qk(p, qh)
            kth = get_qk(6 + p, kc // 4)
            kcol = (kc % 4) * P
            sc = scps.tile([P, 2, QH], f32, tag="sc", name="sc")
            for hi in range(2):
                nc.tensor.matmul(
                    sc[:, hi, :],
                    kth[64 * hi : 64 * hi + 64, kcol : kcol + P],
                    qt[64 * hi : 64 * hi + 64, :],
                    start=True,
                    stop=True,
                    tile_position=(64 * hi, 0),
                )
            if p == 0 and qh == 0:
                make_v(kc, 0)  # strip 0 JIT (AV needs it next iteration)
            run_jit(1)
            ex = exppool.tile([P, 2, QH], bf16, tag="exp", name="ex")
            nc.scalar.activation(ex, sc, FT.Exp, scale=0.125)
            if deferred is not None:
                emit_av(*deferred)
                dp, dqh, dkc, dav, _ = deferred
                if dkc == NCHUNK - 1:
                    normalize(dp, dqh, dav)
            if kc == 0:
                av_cur = [
                    avps.tile([P, 4, VPAD], f32, tag="av", name=f"av{hi}")
                    for hi in range(2)
                ]
            deferred = (p, qh, kc, av_cur, ex)
        emit_av(*deferred)
        normalize(NPAIR - 1, 1, av_cur)

    return nc


def kernel(x: np.ndarray, W_qkv: np.ndarray, b_qkv: np.ndarray) -> np.ndarray:
    nc = build_attention_nc()
    in_maps = [
        {
            "x": np.ascontiguousarray(x[c], dtype=np.float32),
            "W_qkv": np.ascontiguousarray(W_qkv, dtype=np.float32),
            "b_qkv": np.ascontiguousarray(b_qkv, dtype=np.float32),
        }
        for c in range(NCORE)
    ]
    res = run_bass_kernel_spmd(nc, in_maps, core_ids=list(range(NCORE)))
    return np.stack([res.results[c]["out"] for c in range(NCORE)], axis=0)
